# revision 2
# baseline (speedup 1.0000x reference)
"""Det2Mask Trainium kernel.

Reference computation: for each of N=100 detections, gather a 28x28 mask
(mask_outs[idx // 80, idx % 80]), paste it into an HxW (800x800) canvas via
separable bilinear resampling over its box, threshold at 0.5 -> bool masks.

Strategy:
  * The pasted mask is nonzero only within the box (+- half a mask cell), and
    boxes are <= ~300 px wide, so each detection only needs a fixed
    REG x REG (320x320) window of the canvas.
  * Separable bilinear resampling == A_y @ mask @ A_x^T where A_y [REG, 28]
    and A_x [REG, 28] are sparse interpolation matrices that depend only on
    the box. The host builds them (cheap, exact f32 reproduction of the
    reference arithmetic); the device does two small matmuls per detection
    plus the >0.5 threshold, and returns uint8 region rasters.
  * Detections are sharded across the 8 NeuronCores (13 slots per core,
    104 >= 100, spare slots zero-padded). The host pastes the regions into
    the full zero canvas.
"""

import sys

import numpy as np

if "/opt/trn_rl_repo" not in sys.path:
    sys.path.insert(0, "/opt/trn_rl_repo")

NUM_CLASSES = 80
MASK_M = 28
THRESHOLD = 0.5
REG = 320          # per-detection region size (>= max box span + interp margin)
NCORES = 8
DPC = 13           # detection slots per core; 8*13 = 104 >= 100

_CACHE = {}


def _build_bass():
    import concourse.bacc as bacc
    import concourse.mybir as mybir
    import concourse.tile as tile

    f32 = mybir.dt.float32
    u8 = mybir.dt.uint8

    nc = bacc.Bacc(None, target_bir_lowering=False)
    selT_d = nc.dram_tensor("selt", [DPC, MASK_M, MASK_M], f32, kind="ExternalInput")
    axT_d = nc.dram_tensor("axt", [DPC, MASK_M, REG], f32, kind="ExternalInput")
    ayT_d = nc.dram_tensor("ayt", [DPC, MASK_M, REG], f32, kind="ExternalInput")
    out_d = nc.dram_tensor("out", [DPC, REG, REG], u8, kind="ExternalOutput")

    with tile.TileContext(nc) as tc:
        with (
            tc.tile_pool(name="pin", bufs=3) as pin,
            tc.tile_pool(name="ptmp", bufs=3) as ptmp,
            tc.tile_pool(name="ps1", bufs=2, space="PSUM") as pps1,
            tc.tile_pool(name="ps2", bufs=4, space="PSUM") as pps2,
            tc.tile_pool(name="pob", bufs=4) as pob,
        ):
            for d in range(DPC):
                selT_t = pin.tile([MASK_M, MASK_M], f32, tag="selt")
                axT_t = pin.tile([MASK_M, REG], f32, tag="axt")
                ayT_t = pin.tile([MASK_M, REG], f32, tag="ayt")
                nc.sync.dma_start(selT_t[:], selT_d[d])
                nc.sync.dma_start(axT_t[:], axT_d[d])
                nc.sync.dma_start(ayT_t[:], ayT_d[d])

                # tmp = sel @ A_x^T  -> [28, REG]
                pst = pps1.tile([MASK_M, REG], f32, tag="pst")
                nc.tensor.matmul(pst[:], selT_t[:], axT_t[:], start=True, stop=True)
                tmp_t = ptmp.tile([MASK_M, REG], f32, tag="tmp")
                nc.vector.tensor_copy(tmp_t[:], pst[:])

                # out rows tile: A_y[p0:p0+psz, :] @ tmp -> [psz, REG]
                for p0 in range(0, REG, 128):
                    psz = min(128, REG - p0)
                    pso = pps2.tile([psz, REG], f32, tag="pso")
                    nc.tensor.matmul(
                        pso[:], ayT_t[:, p0 : p0 + psz], tmp_t[:],
                        start=True, stop=True,
                    )
                    ot = pob.tile([psz, REG], u8, tag="ot")
                    nc.vector.tensor_scalar(
                        ot[:], pso[:], THRESHOLD, None, mybir.AluOpType.is_gt
                    )
                    nc.sync.dma_start(out_d[d, p0 : p0 + psz, :], ot[:])

    nc.compile()
    return nc


def _get_nc():
    if "nc" not in _CACHE:
        _CACHE["nc"] = _build_bass()
    return _CACHE["nc"]


def _interp_axis(lo, c1, c2, n):
    """f32-exact replica of reference coords/weights for pixels lo..lo+REG-1
    along one axis. Returns (i0, i1, w0, w1) clipped/zeroed like the ref."""
    f = np.float32
    j = (np.arange(lo, lo + REG, dtype=np.float32) + f(0.5) - c1) / (c2 - c1)
    coords = j * f(n) - f(0.5)
    x0 = np.floor(coords)
    t = coords - x0
    x0i = x0.astype(np.int32)
    x1i = x0i + 1
    v0 = ((x0i >= 0) & (x0i < n)).astype(np.float32)
    v1 = ((x1i >= 0) & (x1i < n)).astype(np.float32)
    w0 = (f(1.0) - t) * v0
    w1 = t * v1
    return np.clip(x0i, 0, n - 1), np.clip(x1i, 0, n - 1), w0, w1


def _region_start(c1, c2, limit):
    """Leftmost pixel whose interp weight can be nonzero, clamped so the
    REG-window stays inside [0, limit)."""
    cell = (float(c2) - float(c1)) / MASK_M
    lo = int(np.floor(float(c1) - 0.5 * cell - 0.5)) - 1
    return max(0, min(lo, limit - REG))


def _build_interp_mat(lo, c1, c2):
    """A^T [28, REG] with A[j, m] = w0[j]*(m==i0[j]) + w1[j]*(m==i1[j])."""
    i0, i1, w0, w1 = _interp_axis(lo, c1, c2, MASK_M)
    a = np.zeros((MASK_M, REG), dtype=np.float32)
    jr = np.arange(REG)
    np.add.at(a, (i0, jr), w0)
    np.add.at(a, (i1, jr), w1)
    return a


def _paste_one_host(mask, box, H, W):
    """Full-canvas numpy fallback (exact replica of reference _paste_one)."""
    f = np.float32
    x1, y1, x2, y2 = box
    i0x, i1x, wx0, wx1 = _interp_axis_full(W, x1, x2)
    i0y, i1y, wy0, wy1 = _interp_axis_full(H, y1, y2)
    rows = wy0[:, None] * mask[i0y] + wy1[:, None] * mask[i1y]
    return wx0[None, :] * rows[:, i0x] + wx1[None, :] * rows[:, i1x]


def _interp_axis_full(W, c1, c2):
    f = np.float32
    j = (np.arange(W, dtype=np.float32) + f(0.5) - c1) / (c2 - c1)
    coords = j * f(MASK_M) - f(0.5)
    x0 = np.floor(coords)
    t = coords - x0
    x0i = x0.astype(np.int32)
    x1i = x0i + 1
    v0 = ((x0i >= 0) & (x0i < MASK_M)).astype(np.float32)
    v1 = ((x1i >= 0) & (x1i < MASK_M)).astype(np.float32)
    w0 = (f(1.0) - t) * v0
    w1 = t * v1
    return (
        np.clip(x0i, 0, MASK_M - 1),
        np.clip(x1i, 0, MASK_M - 1),
        w0,
        w1,
    )


def kernel(mask_outs, boxes, scores, indices, class_ids, H, W):
    from concourse.bass_utils import run_bass_kernel_spmd

    H = int(H)
    W = int(W)
    mask_outs = np.asarray(mask_outs)
    boxes = np.asarray(boxes)
    indices = np.asarray(indices)
    N = boxes.shape[0]
    nslots = NCORES * DPC
    assert N <= nslots, f"too many detections: {N} > {nslots}"

    # --- host: gather per-detection masks and build interpolation matrices
    sel = mask_outs[indices // NUM_CLASSES, indices % NUM_CLASSES]  # [N, 28, 28]

    selT = np.zeros((nslots, MASK_M, MASK_M), dtype=np.float32)
    axT = np.zeros((nslots, MASK_M, REG), dtype=np.float32)
    ayT = np.zeros((nslots, MASK_M, REG), dtype=np.float32)
    x0s = np.zeros(N, dtype=np.int64)
    y0s = np.zeros(N, dtype=np.int64)
    fallback = []
    for d in range(N):
        x1, y1, x2, y2 = (float(v) for v in boxes[d])
        span_x = (x2 - x1) * (1.0 + 1.0 / MASK_M) + 3.0
        span_y = (y2 - y1) * (1.0 + 1.0 / MASK_M) + 3.0
        if span_x > REG or span_y > REG or W < REG or H < REG:
            fallback.append(d)
            continue
        x0s[d] = _region_start(x1, x2, W)
        y0s[d] = _region_start(y1, y2, H)
        selT[d] = sel[d].T
        axT[d] = _build_interp_mat(x0s[d], np.float32(x1), np.float32(x2))
        ayT[d] = _build_interp_mat(y0s[d], np.float32(y1), np.float32(y2))

    # --- device: 8-way SPMD over detection slots
    nc = _get_nc()
    in_maps = [
        {
            "selt": np.ascontiguousarray(selT[c * DPC : (c + 1) * DPC]),
            "axt": np.ascontiguousarray(axT[c * DPC : (c + 1) * DPC]),
            "ayt": np.ascontiguousarray(ayT[c * DPC : (c + 1) * DPC]),
        }
        for c in range(NCORES)
    ]
    _CACHE["last_in_maps"] = in_maps
    res = run_bass_kernel_spmd(nc, in_maps, core_ids=list(range(NCORES)))

    # --- host: paste regions into the zero canvas
    masks = np.zeros((N, H, W), dtype=bool)
    for d in range(N):
        if d in fallback:
            pasted = _paste_one_host(
                sel[d].astype(np.float32), boxes[d].astype(np.float32), H, W
            )
            masks[d] = pasted > THRESHOLD
            continue
        c, slot = divmod(d, DPC)
        region = res.results[c]["out"][slot]
        masks[d, y0s[d] : y0s[d] + REG, x0s[d] : x0s[d] + REG] = region.view(bool)

    return masks, scores, class_ids


# revision 4
# speedup vs baseline: 2.3089x; 2.3089x over previous
"""Det2Mask Trainium kernel.

Reference computation: for each of N=100 detections, gather a 28x28 mask
(mask_outs[idx // 80, idx % 80]), paste it into an HxW (800x800) canvas via
separable bilinear resampling over its box, threshold at 0.5 -> bool masks.

Strategy:
  * The pasted mask is nonzero only within the box (+- half a mask cell), and
    boxes are <= ~300 px wide, so each detection only needs a fixed
    REG x REG (320x320) window of the canvas.
  * Separable bilinear resampling == A_y @ mask @ A_x^T where A_y [REG, 28]
    and A_x [REG, 28] are sparse interpolation matrices that depend only on
    the box. The host builds them (exact f32 reproduction of the reference
    arithmetic); the device does two small matmuls per detection plus the
    >0.5 threshold, and returns uint8 region rasters.
  * Detections are sharded across the 8 NeuronCores (13 slots per core,
    104 >= 100, spare slots zero-padded). The host pastes the regions into
    the full zero canvas.
  * Matmuls run in float32r (TF32-like, 1 cycle/row); flips only happen for
    pixels within ~1e-4 of the 0.5 threshold (empirically ~0 of 64M).
"""

import sys

import numpy as np

if "/opt/trn_rl_repo" not in sys.path:
    sys.path.insert(0, "/opt/trn_rl_repo")

NUM_CLASSES = 80
MASK_M = 28
THRESHOLD = 0.5
REG = 320          # per-detection region size (>= max box span + interp margin)
NCORES = 8
DPC = 13           # detection slots per core; 8*13 = 104 >= 100
NCHUNK = 3         # row chunks of REG: 128 + 128 + 64

_CACHE = {}


def _build_bass():
    import concourse.bacc as bacc
    import concourse.mybir as mybir
    import concourse.tile as tile

    f32 = mybir.dt.float32
    f32r = mybir.dt.float32r
    u8 = mybir.dt.uint8
    M = MASK_M

    nc = bacc.Bacc(None, target_bir_lowering=False)
    selt_d = nc.dram_tensor("selt", [M, DPC * M], f32r, kind="ExternalInput")
    axt_d = nc.dram_tensor("axt", [M, DPC * REG], f32r, kind="ExternalInput")
    ayt_d = nc.dram_tensor("ayt", [M, DPC * REG], f32r, kind="ExternalInput")
    out_d = nc.dram_tensor("out", [DPC, 128, NCHUNK * REG], u8, kind="ExternalOutput")

    det_chunks = [(0, 5), (5, 10), (10, DPC)]

    with tile.TileContext(nc) as tc:
        with (
            tc.tile_pool(name="pconst", bufs=1) as pconst,
            tc.tile_pool(name="ptmp", bufs=3) as ptmp,
            tc.tile_pool(name="pob", bufs=3) as pob,
            tc.tile_pool(name="ps1", bufs=2, space="PSUM") as pps1,
            tc.tile_pool(name="ps2", bufs=2, space="PSUM") as pps2,
        ):
            selt_t = pconst.tile([M, DPC * M], f32r)
            axt_t = pconst.tile([M, DPC * REG], f32r)
            ayt_t = pconst.tile([M, DPC * REG], f32r)
            for d0, d1 in det_chunks:
                nc.sync.dma_start(selt_t[:, d0 * M : d1 * M], selt_d[:, d0 * M : d1 * M])
                nc.sync.dma_start(
                    axt_t[:, d0 * REG : d1 * REG], axt_d[:, d0 * REG : d1 * REG]
                )
                nc.sync.dma_start(
                    ayt_t[:, d0 * REG : d1 * REG], ayt_d[:, d0 * REG : d1 * REG]
                )

            for d in range(DPC):
                # tmp = sel @ A_x^T  -> [28, REG]
                pst = pps1.tile([M, REG], f32, tag="pst")
                nc.tensor.matmul(
                    pst[:],
                    selt_t[:, d * M : (d + 1) * M],
                    axt_t[:, d * REG : (d + 1) * REG],
                    start=True,
                    stop=True,
                )
                tmp_t = ptmp.tile([M, REG], f32r, tag="tmp")
                nc.scalar.copy(tmp_t[:], pst[:])
                tmp_r = tmp_t[:]

                # row chunks: A_y[t*128 : t*128+psz, :] @ tmp -> [psz, REG]
                # three matmul outputs parked at 512-f32 strides (bank-aligned)
                pso = pps2.tile([128, NCHUNK * 512], f32, tag="pso")
                for t in range(NCHUNK):
                    p0 = t * 128
                    psz = min(128, REG - p0)
                    nc.tensor.matmul(
                        pso[0:psz, t * 512 : t * 512 + REG],
                        ayt_t[:, d * REG + p0 : d * REG + p0 + psz],
                        tmp_r,
                        start=True,
                        stop=True,
                    )
                ob = pob.tile([128, NCHUNK * REG], u8, tag="ob")
                nc.vector.tensor_scalar(
                    ob[:].rearrange("p (t j) -> p t j", j=REG),
                    pso[:].rearrange("p (t j) -> p t j", j=512)[:, :, 0:REG],
                    THRESHOLD,
                    None,
                    mybir.AluOpType.is_gt,
                )
                nc.sync.dma_start(out_d[d], ob[:])

    nc.compile()
    return nc


def _get_nc():
    if "nc" not in _CACHE:
        _CACHE["nc"] = _build_bass()
    return _CACHE["nc"]


def _interp_axis(lo, c1, c2):
    """f32-exact replica of reference coords/weights for pixels lo..lo+REG-1
    along one axis. Returns (i0, i1, w0, w1) clipped/zeroed like the ref."""
    f = np.float32
    j = (np.arange(lo, lo + REG, dtype=np.float32) + f(0.5) - c1) / (c2 - c1)
    coords = j * f(MASK_M) - f(0.5)
    x0 = np.floor(coords)
    t = coords - x0
    x0i = x0.astype(np.int32)
    x1i = x0i + 1
    v0 = ((x0i >= 0) & (x0i < MASK_M)).astype(np.float32)
    v1 = ((x1i >= 0) & (x1i < MASK_M)).astype(np.float32)
    w0 = (f(1.0) - t) * v0
    w1 = t * v1
    return np.clip(x0i, 0, MASK_M - 1), np.clip(x1i, 0, MASK_M - 1), w0, w1


def _region_start(c1, c2, limit):
    """Leftmost pixel whose interp weight can be nonzero, clamped so the
    REG-window stays inside [0, limit)."""
    cell = (float(c2) - float(c1)) / MASK_M
    lo = int(np.floor(float(c1) - 0.5 * cell - 0.5)) - 1
    return max(0, min(lo, limit - REG))


def _build_interp_mat(lo, c1, c2):
    """A^T [28, REG] with A[j, m] = w0[j]*(m==i0[j]) + w1[j]*(m==i1[j])."""
    i0, i1, w0, w1 = _interp_axis(lo, c1, c2)
    a = np.zeros((MASK_M, REG), dtype=np.float32)
    jr = np.arange(REG)
    np.add.at(a, (i0, jr), w0)
    np.add.at(a, (i1, jr), w1)
    return a


def _interp_axis_full(W, c1, c2):
    f = np.float32
    j = (np.arange(W, dtype=np.float32) + f(0.5) - c1) / (c2 - c1)
    coords = j * f(MASK_M) - f(0.5)
    x0 = np.floor(coords)
    t = coords - x0
    x0i = x0.astype(np.int32)
    x1i = x0i + 1
    v0 = ((x0i >= 0) & (x0i < MASK_M)).astype(np.float32)
    v1 = ((x1i >= 0) & (x1i < MASK_M)).astype(np.float32)
    w0 = (f(1.0) - t) * v0
    w1 = t * v1
    return (
        np.clip(x0i, 0, MASK_M - 1),
        np.clip(x1i, 0, MASK_M - 1),
        w0,
        w1,
    )


def _paste_one_host(mask, box, H, W):
    """Full-canvas numpy fallback (exact replica of reference _paste_one)."""
    x1, y1, x2, y2 = box
    i0x, i1x, wx0, wx1 = _interp_axis_full(W, x1, x2)
    i0y, i1y, wy0, wy1 = _interp_axis_full(H, y1, y2)
    rows = wy0[:, None] * mask[i0y] + wy1[:, None] * mask[i1y]
    return wx0[None, :] * rows[:, i0x] + wx1[None, :] * rows[:, i1x]


def kernel(mask_outs, boxes, scores, indices, class_ids, H, W):
    from concourse.bass_utils import run_bass_kernel_spmd

    H = int(H)
    W = int(W)
    mask_outs = np.asarray(mask_outs)
    boxes = np.asarray(boxes)
    indices = np.asarray(indices)
    N = boxes.shape[0]
    nslots = NCORES * DPC
    assert N <= nslots, f"too many detections: {N} > {nslots}"

    # --- host: gather per-detection masks and build interpolation matrices
    sel = mask_outs[indices // NUM_CLASSES, indices % NUM_CLASSES]  # [N, 28, 28]

    M = MASK_M
    selt = np.zeros((NCORES, M, DPC * M), dtype=np.float32)
    axt = np.zeros((NCORES, M, DPC * REG), dtype=np.float32)
    ayt = np.zeros((NCORES, M, DPC * REG), dtype=np.float32)
    x0s = np.zeros(N, dtype=np.int64)
    y0s = np.zeros(N, dtype=np.int64)
    fallback = set()
    for d in range(N):
        x1, y1, x2, y2 = (float(v) for v in boxes[d])
        span_x = (x2 - x1) * (1.0 + 1.0 / M) + 3.0
        span_y = (y2 - y1) * (1.0 + 1.0 / M) + 3.0
        if span_x > REG or span_y > REG or W < REG or H < REG:
            fallback.add(d)
            continue
        c, s = divmod(d, DPC)
        x0s[d] = _region_start(x1, x2, W)
        y0s[d] = _region_start(y1, y2, H)
        selt[c, :, s * M : (s + 1) * M] = sel[d].T
        axt[c, :, s * REG : (s + 1) * REG] = _build_interp_mat(
            x0s[d], np.float32(x1), np.float32(x2)
        )
        ayt[c, :, s * REG : (s + 1) * REG] = _build_interp_mat(
            y0s[d], np.float32(y1), np.float32(y2)
        )

    # --- device: 8-way SPMD over detection slots
    nc = _get_nc()
    in_maps = [
        {"selt": selt[c], "axt": axt[c], "ayt": ayt[c]} for c in range(NCORES)
    ]
    _CACHE["last_in_maps"] = in_maps
    res = run_bass_kernel_spmd(nc, in_maps, core_ids=list(range(NCORES)))

    # --- host: paste regions into the zero canvas
    masks = np.zeros((N, H, W), dtype=bool)
    for d in range(N):
        if d in fallback:
            pasted = _paste_one_host(
                sel[d].astype(np.float32), boxes[d].astype(np.float32), H, W
            )
            masks[d] = pasted > THRESHOLD
            continue
        c, s = divmod(d, DPC)
        raw = res.results[c]["out"][s]  # [128, 3*REG] u8
        chunks = raw.reshape(128, NCHUNK, REG)
        region = np.concatenate(
            [chunks[:, 0, :], chunks[:, 1, :], chunks[: REG - 256, 2, :]], axis=0
        )
        masks[d, y0s[d] : y0s[d] + REG, x0s[d] : x0s[d] + REG] = region.view(bool)

    return masks, scores, class_ids
